# revision 22
# baseline (speedup 1.0000x reference)
"""Trainium2 Bass kernel for nn_MoEEncoderLayer_3504693313985.

Sharding: token-parallel over 8 cores. Core c handles batch b=c//4, query rows
q0=(c%4)*512 .. q0+512. Each core computes: K/V projections for its full batch
(2048 tokens), Q for its 512 rows, full attention + softmax (attn probs are a
kernel output), LN1, the gate (top-2 of 8 experts, renormalized softmax), all
8 routed experts computed densely on its 512 tokens (mathematically exact:
non-top-2 weights are zero), the general expert, residual + LN2.

No collectives; outputs are disjoint per core and assembled on host.

Matmuls run in float32r (TF32-like, ~1.6e-4 rel err, full PE speed). The
attention probabilities are transposed head-by-head via PE transposes in bf16
to feed the attn@V contraction (probs also written to HBM in f32).
"""
import functools
import os
import numpy as np

import concourse.bass as bass
import concourse.mybir as mybir
import concourse.tile as tile
from concourse import bacc
from concourse.bass_utils import run_bass_kernel_spmd
from concourse.masks import make_identity

F32 = mybir.dt.float32
F32R = mybir.dt.float32r
BF16 = mybir.dt.bfloat16
AX = mybir.AxisListType
OP = mybir.AluOpType
AF = mybir.ActivationFunctionType

B, L, D, H, E, K, DF = 2, 2048, 512, 8, 8, 2, 2048
HD = D // H            # 64
P = 128
NCORES = 8
QO = L * B // NCORES   # 512 tokens per core
DT = D // P            # 4
QT = QO // P           # 4
LT = L // P            # 16
FT = DF // P           # 16
EPS_GATE = 1e-9
EPS_LN = 1e-5


def _bcast_ap(ap, p=P):
    """Partition-broadcast a DRAM vector AP across p partitions."""
    return bass.AP(tensor=ap.tensor, offset=ap.offset, ap=[[0, p], *ap.ap])


def build_nc():
    # debug stage-skip hook (always empty in production; KSKIP env intentionally
    # not honored so a stray env var can never truncate the kernel)
    nc = bacc.Bacc(None, target_bir_lowering=False)
    build_nc._skip = set()

    def inp(name, shape):
        return nc.dram_tensor(name, shape, F32, kind="ExternalInput")

    x_batch = inp("x_batch", [L, D])
    x_own = inp("x_own", [QO, D])
    Wq, Wk, Wv, Wo = (inp(n, [D, D]) for n in ["Wq", "Wk", "Wv", "Wo"])
    bq, bk, bv, bo = (inp(n, [D]) for n in ["bq", "bk", "bv", "bo"])
    ln1_g, ln1_b = inp("ln1_g", [D]), inp("ln1_b", [D])
    ln2_g, ln2_b = inp("ln2_g", [D]), inp("ln2_b", [D])
    gW1, gb1 = inp("gW1", [D, D]), inp("gb1", [D])
    gW2, gb2 = inp("gW2", [D, E]), inp("gb2", [E])
    eW1 = nc.dram_tensor("eW1", [E, D, DF], BF16, kind="ExternalInput")
    eW2 = nc.dram_tensor("eW2", [E, DF, D], BF16, kind="ExternalInput")
    geW1 = nc.dram_tensor("geW1", [D, DF], BF16, kind="ExternalInput")
    geW2 = nc.dram_tensor("geW2", [DF, D], BF16, kind="ExternalInput")
    eb1, eb2 = inp("eb1", [E, DF]), inp("eb2", [E, D])
    geb1, geb2 = inp("geb1", [DF]), inp("geb2", [D])

    out_y = nc.dram_tensor("out_y", [QO, D], F32, kind="ExternalOutput")
    out_attn = nc.dram_tensor("out_attn", [H, QO, L], F32, kind="ExternalOutput")

    def wmat_into(pool, w, tag, name):
        t = pool.tile([P, DT, D], F32R, tag=tag, name=name)
        nc.sync.dma_start(
            t, w.ap().rearrange("(kt p) d -> p kt d", p=P).bitcast(F32R))
        return t

    with tile.TileContext(nc) as tc:
        with tc.tile_pool(name="consts", bufs=1) as consts:
            # ---- constants (~7KB/part) ----
            ident_f = consts.tile([P, P], F32)
            make_identity(nc, ident_f)
            ident_bf = consts.tile([P, P], BF16)
            nc.vector.tensor_copy(ident_bf, ident_f)
            ones_f = consts.tile([1, P], F32)
            nc.vector.memset(ones_f, 1.0)
            ones_r = consts.tile([1, P], F32R)
            nc.vector.tensor_copy(ones_r, ones_f)
            eps_t = consts.tile([P, 1], F32)
            nc.vector.memset(eps_t, EPS_LN)

            def colv(v, n_t):
                t = consts.tile([P, n_t], F32, tag=f"colv_{v.name}")
                nc.sync.dma_start(t, v.ap().rearrange("(t p) -> p t", p=P))
                return t

            bq_c, bk_c, bo_c = colv(bq, DT), colv(bk, DT), colv(bo, DT)
            gb1_c = colv(gb1, DT)
            geb1_c = colv(geb1, FT)
            eb1_c = consts.tile([P, E, FT], F32)
            nc.sync.dma_start(eb1_c, eb1.ap().rearrange("e (t p) -> p e t", p=P))

            bv_row = consts.tile([1, D], F32R)
            nc.sync.dma_start(bv_row, bv.ap().unsqueeze(0).bitcast(F32R))
            gb2_row = consts.tile([1, E], F32R)
            nc.sync.dma_start(gb2_row, gb2.ap().unsqueeze(0).bitcast(F32R))

            def bc(pool, v):
                t = pool.tile([P, D], F32, tag=f"bc_{v.name}")
                nc.sync.dma_start(t, _bcast_ap(v.ap()))
                return t

            gW2_s = consts.tile([P, DT, E], F32R)
            nc.sync.dma_start(
                gW2_s, gW2.ap().rearrange("(kt p) d -> p kt d", p=P).bitcast(F32R))

            with tc.tile_pool(name="perB", bufs=1) as perB:
                xt_nat = perB.tile([P, QT, D], F32)
                xtT = perB.tile([P, DT, QO], F32R)
                xtT_bf = perB.tile([P, DT, QO], BF16)
                w_sb = perB.tile([P, QT, E], F32)
                moe_acc = perB.tile([P, QT, D], F32)

                with tc.tile_pool(name="wog", bufs=1) as wog_p:
                    Wo_s = wmat_into(wog_p, Wo, "Wo", "Wo_s")
                    gW1_s = wmat_into(wog_p, gW1, "gW1", "gW1_s")
                    self_attn_and_ln1(
                        nc, tc, locals())
                    gate(nc, tc, locals())
                experts(nc, tc, locals())

    nc.compile()
    return nc


def self_attn_and_ln1(nc, tc, env):
    g = env
    consts = g["consts"]
    ident_f, ident_bf = g["ident_f"], g["ident_bf"]
    ones_r, bv_row = g["ones_r"], g["bv_row"]
    bq_c, bk_c, bo_c, eps_t = g["bq_c"], g["bk_c"], g["bo_c"], g["eps_t"]
    Wo_s = g["Wo_s"]
    x_batch, x_own = g["x_batch"], g["x_own"]
    Wq, Wk, Wv = g["Wq"], g["Wk"], g["Wv"]
    ln1_g, ln1_b = g["ln1_g"], g["ln1_b"]
    out_attn = g["out_attn"]
    xt_nat, xtT = g["xt_nat"], g["xtT"]
    xtT_bf = g["xtT_bf"]
    wmat = g["wmat_into"]
    bcf = g["bc"]

    with tc.tile_pool(name="perA", bufs=1) as perA:
        xo_nat = perA.tile([P, QT, D], F32)
        nc.sync.dma_start(xo_nat,
                          x_own.ap().rearrange("(qt p) d -> p qt d", p=P))
        kT = perA.tile([P, DT, L], F32R)
        V_bf = perA.tile([P, LT, D], BF16)
        qT = perA.tile([P, DT, QO], F32R)

        # ============ stage 1+2: xT, xoT; kT, V, qT ============
        with (
            tc.tile_pool(name="wqkv", bufs=2) as wqkv_p,
            tc.tile_pool(name="xTp", bufs=1) as xTp,
            tc.tile_pool(name="s1in", bufs=2) as s1in,
            tc.tile_pool(name="ps_tp1", bufs=3, space="PSUM") as ps_tp1,
            tc.tile_pool(name="ps_mm1", bufs=4, space="PSUM") as ps_mm1,
        ):
            xT = xTp.tile([P, DT, L], F32R)
            xoT = xTp.tile([P, DT, QO], F32R)
            for lg in range(8):
                xin = s1in.tile([P, 2, D], F32, tag="xin")
                nc.sync.dma_start(
                    xin, x_batch.ap().rearrange("(lt p) d -> p lt d", p=P)
                    [:, lg * 2:(lg + 1) * 2, :])
                for j in range(2):
                    lt = lg * 2 + j
                    ps = ps_tp1.tile([P, 4 * P], F32, tag="tp1")
                    for dt in range(DT):
                        nc.tensor.transpose(
                            ps[:, dt * P:(dt + 1) * P],
                            xin[:, j, dt * P:(dt + 1) * P], ident_f)
                    nc.vector.tensor_copy(
                        xT[:, :, lt * P:(lt + 1) * P],
                        ps.rearrange("p (a b) -> p a b", a=DT))
            for qc in range(QT):
                ps = ps_tp1.tile([P, 4 * P], F32, tag="tp1")
                for dt in range(DT):
                    nc.tensor.transpose(
                        ps[:, dt * P:(dt + 1) * P],
                        xo_nat[:, qc, dt * P:(dt + 1) * P], ident_f)
                nc.vector.tensor_copy(
                    xoT[:, :, qc * P:(qc + 1) * P],
                    ps.rearrange("p (a b) -> p a b", a=DT))

            Wk_s = wmat(wqkv_p, Wk, "wqkv", "Wk_s")
            for dt in range(DT):
                for lc in range(L // 512):
                    ps = ps_mm1.tile([P, 512], F32, tag="mm1")
                    for kt in range(DT):
                        nc.tensor.matmul(
                            ps, Wk_s[:, kt, dt * P:(dt + 1) * P],
                            xT[:, kt, lc * 512:(lc + 1) * 512],
                            start=(kt == 0), stop=(kt == DT - 1))
                    nc.vector.tensor_scalar(
                        kT[:, dt, lc * 512:(lc + 1) * 512], ps,
                        bk_c[:, dt:dt + 1], None, OP.add)
            Wv_s = wmat(wqkv_p, Wv, "wqkv", "Wv_s")
            for lt in range(LT):
                ps = ps_mm1.tile([P, 512], F32, tag="mm1")
                nc.tensor.matmul(ps, ones_r, bv_row, start=True, stop=False,
                                 skip_group_check=True)
                for kt in range(DT):
                    nc.tensor.matmul(
                        ps, xT[:, kt, lt * P:(lt + 1) * P], Wv_s[:, kt, :],
                        start=False, stop=(kt == DT - 1), skip_group_check=True)
                nc.vector.tensor_copy(V_bf[:, lt, :], ps)
            Wq_s = wmat(wqkv_p, Wq, "wqkv", "Wq_s")
            for dt in range(DT):
                ps = ps_mm1.tile([P, 512], F32, tag="mm1")
                for kt in range(DT):
                    nc.tensor.matmul(
                        ps, Wq_s[:, kt, dt * P:(dt + 1) * P], xoT[:, kt, :],
                        start=(kt == 0), stop=(kt == DT - 1))
                nc.vector.tensor_scalar(
                    qT[:, dt, :], ps, bq_c[:, dt:dt + 1], None, OP.add)

        with tc.tile_pool(name="aoT_p", bufs=1) as aoT_p:
            aoT = aoT_p.tile([P, DT, QO], F32R)

            # ============ stage 3+4: attention per head ============
            with (
                tc.tile_pool(name="exps", bufs=2) as exps_p,
                tc.tile_pool(name="attnf", bufs=2) as attnf_p,
                tc.tile_pool(name="attnb", bufs=2) as attnb_p,
                tc.tile_pool(name="expTp", bufs=2) as expT_p,
                tc.tile_pool(name="small3", bufs=8) as small3,
                tc.tile_pool(name="ps_s", bufs=1, space="PSUM") as ps_s,
                tc.tile_pool(name="ps_t3", bufs=2, space="PSUM") as ps_t3,
                tc.tile_pool(name="ps_av", bufs=1, space="PSUM") as ps_av,
            ):
                for h in range(H if "attn" not in build_nc._skip else 0):
                    hp, hr = h // 2, (h % 2) * HD
                    expT = expT_p.tile([P, LT, QO], BF16, tag="expT")
                    sums_h = small3.tile([P, QT], F32, tag="sums",
                                         name=f"sums_{h}")
                    # natural scores -> exp (f32) -> normalized attn out,
                    # interleaved with transposed-score chunks for pipelining
                    for qc in range(QT):
                        for ltc in range(qc * 2, qc * 2 + 2):
                            ps = ps_t3.tile([P, 1024], F32, tag="sct")
                            for j in range(2):
                                lt = ltc * 2 + j
                                nc.tensor.matmul(
                                    ps[:, j * 512:(j + 1) * 512],
                                    kT[hr:hr + HD, hp, lt * P:(lt + 1) * P],
                                    qT[hr:hr + HD, hp, :],
                                    start=True, stop=True)
                            nc.scalar.activation(
                                expT[:, ltc * 2:(ltc + 1) * 2, :].rearrange(
                                    "p a b -> p (a b)"),
                                ps, AF.Exp, scale=0.125)
                        qs = qT[hr:hr + HD, hp, qc * P:(qc + 1) * P]
                        halfsum = []
                        exp_sb = exps_p.tile([P, L], F32, tag="exps")
                        for half in range(2):
                            ps = ps_s.tile([P, 1024], F32, tag="sc")
                            for j in range(2):
                                lc = half * 2 + j
                                nc.tensor.matmul(
                                    ps[:, j * 512:(j + 1) * 512], qs,
                                    kT[hr:hr + HD, hp, lc * 512:(lc + 1) * 512],
                                    start=True, stop=True)
                            s = small3.tile([P, 1], F32, tag="s3")
                            nc.scalar.activation(
                                exp_sb[:, half * 1024:(half + 1) * 1024], ps,
                                AF.Exp, scale=0.125, accum_out=s)
                            halfsum.append(s)
                        nc.vector.tensor_tensor(
                            sums_h[:, qc:qc + 1], halfsum[0], halfsum[1],
                            OP.add)
                        rcp = small3.tile([P, 1], F32, tag="s3")
                        nc.vector.reciprocal(rcp, sums_h[:, qc:qc + 1])
                        if "attnout" not in build_nc._skip:
                            attn_f = attnf_p.tile([P, L], F32, tag="attnf")
                            nc.vector.tensor_scalar(attn_f, exp_sb, rcp, None,
                                                    OP.mult)
                            nc.sync.dma_start(
                                out_attn.ap()[h, qc * P:(qc + 1) * P, :],
                                attn_f)
                    # reciprocal softmax-denominator row [1, QO]
                    psr = ps_t3.tile([P, 1024], F32, tag="sct",
                                     name=f"psr_{h}")
                    for qc in range(QT):
                        nc.tensor.transpose(
                            psr[0:1, qc * P:(qc + 1) * P],
                            sums_h[:, qc:qc + 1], ident_f)
                    rcp_row = attnb_p.tile([1, QO], F32, tag="rcpr",
                                           name=f"rcpr_{h}")
                    nc.vector.reciprocal(rcp_row, psr[0:1, 0:QO])
                    rcp_bc = attnb_p.tile([HD, QO], F32, tag="rcpbc",
                                          name=f"rcpbc_{h}")
                    nc.gpsimd.partition_broadcast(rcp_bc, rcp_row)
                    # attn @ V (unnormalized), then scale columns by 1/sum
                    ps = ps_av.tile([HD, 512], F32, tag="av")
                    for lt in range(LT):
                        nc.tensor.matmul(
                            ps, V_bf[:, lt, h * HD:(h + 1) * HD],
                            expT[:, lt, :],
                            start=(lt == 0), stop=(lt == LT - 1))
                    nc.vector.tensor_tensor(
                        aoT[hr:hr + HD, hp, :], ps, rcp_bc, OP.mult)

            # ============ stage 5: out-proj, residual, LN1 ============
            with (
                tc.tile_pool(name="s5", bufs=4) as s5,
                tc.tile_pool(name="s5ln", bufs=1) as s5ln,
                tc.tile_pool(name="nxTp", bufs=1) as nxT_p,
                tc.tile_pool(name="ps_mm5", bufs=2, space="PSUM") as ps_mm5,
                tc.tile_pool(name="ps_tp5", bufs=2, space="PSUM") as ps_tp5,
            ):
                ln1g_b, ln1b_b = bcf(s5ln, ln1_g), bcf(s5ln, ln1_b)
                nxT = nxT_p.tile([P, DT, QO], F32)
                for dt in range(DT):
                    ps = ps_mm5.tile([P, 512], F32, tag="mm5")
                    for kt in range(DT):
                        nc.tensor.matmul(
                            ps, Wo_s[:, kt, dt * P:(dt + 1) * P],
                            aoT[:, kt, :],
                            start=(kt == 0), stop=(kt == DT - 1))
                    nc.vector.tensor_scalar(
                        nxT[:, dt, :], ps, bo_c[:, dt:dt + 1], None, OP.add)
                for qc in range(QT):
                    ps = ps_tp5.tile([P, 4 * P], F32, tag="tp5")
                    for dt in range(DT):
                        nc.tensor.transpose(
                            ps[:, dt * P:(dt + 1) * P],
                            nxT[:, dt, qc * P:(qc + 1) * P], ident_f)
                    r_nat = s5.tile([P, D], F32, tag="rnat")
                    nc.vector.tensor_tensor(
                        r_nat.rearrange("p (a b) -> p a b", a=DT),
                        ps.rearrange("p (a b) -> p a b", a=DT),
                        xo_nat[:, qc].rearrange("p (a b) -> p a b", a=DT),
                        OP.add)
                    stats = s5.tile([P, nc.vector.BN_STATS_DIM], F32, tag="st5")
                    nc.vector.bn_stats(stats, r_nat)
                    mv = s5.tile([P, nc.vector.BN_AGGR_DIM], F32, tag="mv5")
                    nc.vector.bn_aggr(mv, stats)
                    std = s5.tile([P, 1], F32, tag="sd5")
                    nc.scalar.activation(std, mv[:, 1:2], AF.Sqrt, bias=eps_t)
                    rstd = s5.tile([P, 1], F32, tag="rs5")
                    nc.vector.reciprocal(rstd, std)
                    tmp = s5.tile([P, D], F32, tag="tmp5")
                    nc.vector.tensor_scalar(
                        tmp, r_nat, mv[:, 0:1], rstd, OP.subtract, OP.mult)
                    nc.vector.tensor_tensor(tmp, tmp, ln1g_b, OP.mult)
                    nc.vector.tensor_tensor(xt_nat[:, qc], tmp, ln1b_b, OP.add)
                    ps2 = ps_tp5.tile([P, 4 * P], F32, tag="tp5b")
                    for dt in range(DT):
                        nc.tensor.transpose(
                            ps2[:, dt * P:(dt + 1) * P],
                            xt_nat[:, qc, dt * P:(dt + 1) * P], ident_f)
                    nc.vector.tensor_copy(
                        xtT[:, :, qc * P:(qc + 1) * P],
                        ps2.rearrange("p (a b) -> p a b", a=DT))
                    nc.vector.tensor_copy(
                        xtT_bf[:, :, qc * P:(qc + 1) * P],
                        ps2.rearrange("p (a b) -> p a b", a=DT))


def gate(nc, tc, env):
    g = env
    ones_r, gb2_row = g["ones_r"], g["gb2_row"]
    gb1_c, gW1_s, gW2_s = g["gb1_c"], g["gW1_s"], g["gW2_s"]
    xtT, w_sb = g["xtT"], g["w_sb"]

    with (
        tc.tile_pool(name="ghTp", bufs=1) as ghT_p,
        tc.tile_pool(name="s6", bufs=6) as s6,
        tc.tile_pool(name="ps_g", bufs=2, space="PSUM") as ps_g,
        tc.tile_pool(name="ps_l", bufs=2, space="PSUM") as ps_l,
    ):
        ghT = ghT_p.tile([P, DT, QO], F32R)
        for dt in range(DT):
            ps = ps_g.tile([P, 512], F32, tag="g6")
            for kt in range(DT):
                nc.tensor.matmul(
                    ps, gW1_s[:, kt, dt * P:(dt + 1) * P], xtT[:, kt, :],
                    start=(kt == 0), stop=(kt == DT - 1))
            nc.scalar.activation(
                ghT[:, dt, :], ps, AF.Gelu_apprx_tanh,
                bias=gb1_c[:, dt:dt + 1])
        for qc in range(QT):
            ps = ps_l.tile([P, E], F32, tag="l6")
            nc.tensor.matmul(ps, ones_r, gb2_row, start=True, stop=False,
                             skip_group_check=True)
            for kt in range(DT):
                nc.tensor.matmul(
                    ps, ghT[:, kt, qc * P:(qc + 1) * P], gW2_s[:, kt, :],
                    start=False, stop=(kt == DT - 1), skip_group_check=True)
            lg = s6.tile([P, E], F32, tag="lg")
            nc.vector.tensor_copy(lg, ps)
            m1 = s6.tile([P, 1], F32, tag="m1")
            nc.vector.reduce_max(m1, lg, axis=AX.X)
            mask1 = s6.tile([P, E], F32, tag="mk1")
            nc.vector.tensor_scalar(mask1, lg, m1, None, OP.is_ge)
            neg = s6.tile([P, E], F32, tag="neg")
            nc.vector.tensor_scalar(neg, mask1, -1e30, None, OP.mult)
            l2 = s6.tile([P, E], F32, tag="l2")
            nc.vector.tensor_tensor(l2, lg, neg, OP.add)
            m2 = s6.tile([P, 1], F32, tag="m2")
            nc.vector.reduce_max(m2, l2, axis=AX.X)
            mask2 = s6.tile([P, E], F32, tag="mk2")
            nc.vector.tensor_scalar(mask2, l2, m2, None, OP.is_ge)
            mask = s6.tile([P, E], F32, tag="mk")
            nc.vector.tensor_tensor(mask, mask1, mask2, OP.add)
            ex = s6.tile([P, E], F32, tag="ex")
            es = s6.tile([P, 1], F32, tag="es")
            nc.scalar.activation(ex, lg, AF.Exp, accum_out=es)
            pm = s6.tile([P, E], F32, tag="pm")
            nc.vector.tensor_tensor(pm, ex, mask, OP.mult)
            esr = s6.tile([P, 1], F32, tag="esr")
            nc.vector.reciprocal(esr, es)
            w0 = s6.tile([P, E], F32, tag="w0")
            nc.vector.tensor_scalar(w0, pm, esr, None, OP.mult)
            ws = s6.tile([P, 1], F32, tag="ws")
            nc.vector.reduce_sum(ws, w0, axis=AX.X)
            wse = s6.tile([P, 1], F32, tag="wse")
            nc.vector.tensor_scalar(wse, ws, EPS_GATE, None, OP.add)
            wsr = s6.tile([P, 1], F32, tag="wsr")
            nc.vector.reciprocal(wsr, wse)
            nc.vector.tensor_scalar(w_sb[:, qc], w0, wsr, None, OP.mult)


def experts(nc, tc, env):
    g = env
    ones_r, eps_t = g["ones_r"], g["eps_t"]
    eb1_c, geb1_c = g["eb1_c"], g["geb1_c"]
    eb2, geb2 = g["eb2"], g["geb2"]
    ln2_g, ln2_b = g["ln2_g"], g["ln2_b"]
    bcf = g["bc"]
    eW1, eW2, geW1, geW2 = g["eW1"], g["eW2"], g["geW1"], g["geW2"]
    xtT = g["xtT_bf"]
    xt_nat, w_sb, moe_acc = g["xt_nat"], g["w_sb"], g["moe_acc"]
    out_y = g["out_y"]

    with (
        tc.tile_pool(name="ebias", bufs=1) as ebias_p,
        tc.tile_pool(name="hTp", bufs=2) as hT_p,
        tc.tile_pool(name="ew1", bufs=3) as ew1_p,
        tc.tile_pool(name="ew2", bufs=3) as ew2_p,
        tc.tile_pool(name="ytmp", bufs=4) as ytmp_p,
        tc.tile_pool(name="fin", bufs=4) as fin_p,
        tc.tile_pool(name="ps_h", bufs=2, space="PSUM") as ps_h,
        tc.tile_pool(name="ps_y", bufs=4, space="PSUM") as ps_y,
    ):
        eb2_rows = ebias_p.tile([1, E, D], F32R)
        nc.sync.dma_start(eb2_rows, eb2.ap().unsqueeze(0).bitcast(F32R))
        geb2_row = ebias_p.tile([1, D], F32R)
        nc.sync.dma_start(geb2_row, geb2.ap().unsqueeze(0).bitcast(F32R))
        ln2g_b, ln2b_b = bcf(ebias_p, ln2_g), bcf(ebias_p, ln2_b)

        def expert(e):
            if e < E:
                w1, w2 = eW1.ap()[e], eW2.ap()[e]
                b1c, b2row = eb1_c[:, e], eb2_rows[:, e]
            else:
                w1, w2 = geW1.ap(), geW2.ap()
                b1c, b2row = geb1_c, geb2_row
            hT = hT_p.tile([P, FT, QO], BF16, tag="hT", name=f"hT_{e}")
            for fc in range(4):
                w1_s = ew1_p.tile([P, DT, 512], BF16, tag="w1",
                                  name=f"w1_{e}_{fc}")
                nc.sync.dma_start(
                    w1_s,
                    w1.rearrange("(kt p) f -> p kt f", p=P)
                    [:, :, fc * 512:(fc + 1) * 512])
                for j2 in range(2):
                    df0 = fc * 4 + j2 * 2
                    ps = ps_h.tile([P, 1024], F32, tag="h7",
                                   name=f"h7_{e}_{df0}")
                    for j in range(2):
                        for kt in range(DT):
                            nc.tensor.matmul(
                                ps[:, j * 512:(j + 1) * 512],
                                w1_s[:, kt,
                                     (j2 * 2 + j) * P:(j2 * 2 + j + 1) * P],
                                xtT[:, kt, :],
                                start=(kt == 0), stop=(kt == DT - 1))
                    for j in range(2):
                        df = df0 + j
                        nc.scalar.activation(
                            hT[:, df, :], ps[:, j * 512:(j + 1) * 512],
                            AF.Gelu_apprx_tanh, bias=b1c[:, df:df + 1])
            ps_ys = [ps_y.tile([P, D], F32, tag="y7", name=f"y7_{e}_{i}")
                     for i in range(QT)]
            for qc in range(QT):
                nc.tensor.matmul(ps_ys[qc], ones_r, b2row,
                                 start=True, stop=False,
                                 skip_group_check=True)
            for c4 in range(4):
                w2_s = ew2_p.tile([P, 4, D], BF16, tag="w2",
                                  name=f"w2_{e}_{c4}")
                nc.sync.dma_start(
                    w2_s,
                    w2.rearrange("(kt p) d -> p kt d", p=P)
                    [:, c4 * 4:(c4 + 1) * 4, :])
                for j4 in range(4):
                    df = c4 * 4 + j4
                    for qc in range(QT):
                        nc.tensor.matmul(
                            ps_ys[qc], hT[:, df, qc * P:(qc + 1) * P],
                            w2_s[:, j4, :],
                            start=False, stop=(df == FT - 1),
                            skip_group_check=True)
            for qc in range(QT):
                ps = ps_ys[qc]
                if e < E:
                    tmp = ytmp_p.tile([P, D], F32, tag="yt",
                                      name=f"yt_{e}_{qc}")
                    nc.vector.tensor_scalar(
                        tmp, ps, w_sb[:, qc, e:e + 1], None, OP.mult)
                    if e == 0:
                        nc.gpsimd.tensor_copy(moe_acc[:, qc], tmp)
                    else:
                        nc.gpsimd.tensor_tensor(
                            moe_acc[:, qc], moe_acc[:, qc], tmp, OP.add)
                else:
                    moe_bf = fin_p.tile([P, D], BF16, tag="mbf",
                                        name=f"mbf_{qc}")
                    nc.vector.tensor_copy(moe_bf, moe_acc[:, qc])
                    t1 = fin_p.tile([P, D], F32, tag="t1", name=f"t1_{qc}")
                    nc.vector.tensor_tensor(t1, ps, moe_bf, OP.add)
                    t2 = fin_p.tile([P, D], F32, tag="t2", name=f"t2_{qc}")
                    nc.vector.tensor_tensor(t2, t1, xt_nat[:, qc], OP.add)
                    stats = fin_p.tile([P, nc.vector.BN_STATS_DIM], F32,
                                       tag="st8", name=f"st8_{qc}")
                    nc.vector.bn_stats(stats, t2)
                    mv = fin_p.tile([P, nc.vector.BN_AGGR_DIM], F32,
                                    tag="mv8", name=f"mv8_{qc}")
                    nc.vector.bn_aggr(mv, stats)
                    std = fin_p.tile([P, 1], F32, tag="sd8", name=f"sd8_{qc}")
                    nc.scalar.activation(std, mv[:, 1:2], AF.Sqrt, bias=eps_t)
                    rstd = fin_p.tile([P, 1], F32, tag="rs8",
                                      name=f"rs8_{qc}")
                    nc.vector.reciprocal(rstd, std)
                    on = fin_p.tile([P, D], F32, tag="on", name=f"on_{qc}")
                    nc.vector.tensor_scalar(
                        on, t2, mv[:, 0:1], rstd, OP.subtract, OP.mult)
                    nc.vector.tensor_tensor(on, on, ln2g_b, OP.mult)
                    nc.vector.tensor_tensor(on, on, ln2b_b, OP.add)
                    nc.sync.dma_start(out_y.ap()[qc * P:(qc + 1) * P, :], on)

        import __main__
        skip = getattr(build_nc, "_skip", set())
        n_e = 0 if "experts" in skip else (E + 1)
        for e in range(n_e):
            expert(e)


@functools.lru_cache(maxsize=1)
def _get_nc():
    return build_nc()


_WNAMES = ["Wq", "bq", "Wk", "bk", "Wv", "bv", "Wo", "bo", "ln1_g", "ln1_b",
           "gW1", "gb1", "gW2", "gb2", "eW1", "eb1", "eW2", "eb2",
           "geW1", "geb1", "geW2", "geb2", "ln2_g", "ln2_b"]


def kernel(**inputs):
    nc = _get_nc()
    import ml_dtypes
    x = np.ascontiguousarray(np.asarray(inputs["x"], np.float32))
    shared = {}
    for n in _WNAMES:
        a = np.asarray(inputs[n], np.float32)
        if n in ("eW1", "eW2", "geW1", "geW2"):
            a = a.astype(ml_dtypes.bfloat16)
        shared[n] = np.ascontiguousarray(a)
    in_maps = []
    for c in range(NCORES):
        b, q0 = c // (NCORES // B), (c % (NCORES // B)) * QO
        m = dict(shared)
        m["x_batch"] = x[b]
        m["x_own"] = np.ascontiguousarray(x[b, q0:q0 + QO])
        in_maps.append(m)

    res = run_bass_kernel_spmd(nc, in_maps, core_ids=list(range(NCORES)))

    out = np.empty((B, L, D), np.float32)
    attn = np.empty((B, H, L, L), np.float32)
    for c in range(NCORES):
        b, q0 = c // (NCORES // B), (c % (NCORES // B)) * QO
        out[b, q0:q0 + QO] = res.results[c]["out_y"]
        attn[b, :, q0:q0 + QO, :] = res.results[c]["out_attn"]
    return out, attn, np.float32(0.0)


# revision 23
# speedup vs baseline: 1.0088x; 1.0088x over previous
"""Trainium2 Bass kernel for nn_MoEEncoderLayer_3504693313985.

Sharding: token-parallel over 8 cores. Core c handles batch b=c//4, query rows
q0=(c%4)*512 .. q0+512. Each core computes: K/V projections for its full batch
(2048 tokens), Q for its 512 rows, full attention + softmax (attn probs are a
kernel output), LN1, the gate (top-2 of 8 experts, renormalized softmax), all
8 routed experts computed densely on its 512 tokens (mathematically exact:
non-top-2 weights are zero), the general expert, residual + LN2.

No collectives; outputs are disjoint per core and assembled on host.

Matmuls run in float32r (TF32-like, ~1.6e-4 rel err, full PE speed). The
attention probabilities are transposed head-by-head via PE transposes in bf16
to feed the attn@V contraction (probs also written to HBM in f32).
"""
import functools
import os
import numpy as np

import concourse.bass as bass
import concourse.mybir as mybir
import concourse.tile as tile
from concourse import bacc
from concourse.bass_utils import run_bass_kernel_spmd
from concourse.masks import make_identity

F32 = mybir.dt.float32
F32R = mybir.dt.float32r
BF16 = mybir.dt.bfloat16
AX = mybir.AxisListType
OP = mybir.AluOpType
AF = mybir.ActivationFunctionType

B, L, D, H, E, K, DF = 2, 2048, 512, 8, 8, 2, 2048
HD = D // H            # 64
P = 128
NCORES = 8
QO = L * B // NCORES   # 512 tokens per core
DT = D // P            # 4
QT = QO // P           # 4
LT = L // P            # 16
FT = DF // P           # 16
EPS_GATE = 1e-9
EPS_LN = 1e-5


def _bcast_ap(ap, p=P):
    """Partition-broadcast a DRAM vector AP across p partitions."""
    return bass.AP(tensor=ap.tensor, offset=ap.offset, ap=[[0, p], *ap.ap])


def build_nc():
    # debug stage-skip hook (always empty in production; KSKIP env intentionally
    # not honored so a stray env var can never truncate the kernel)
    nc = bacc.Bacc(None, target_bir_lowering=False)
    build_nc._skip = set()

    def inp(name, shape):
        return nc.dram_tensor(name, shape, F32, kind="ExternalInput")

    x_batch = inp("x_batch", [L, D])
    x_own = inp("x_own", [QO, D])
    Wq, Wk, Wv, Wo = (inp(n, [D, D]) for n in ["Wq", "Wk", "Wv", "Wo"])
    bq, bk, bv, bo = (inp(n, [D]) for n in ["bq", "bk", "bv", "bo"])
    ln1_g, ln1_b = inp("ln1_g", [D]), inp("ln1_b", [D])
    ln2_g, ln2_b = inp("ln2_g", [D]), inp("ln2_b", [D])
    gW1, gb1 = inp("gW1", [D, D]), inp("gb1", [D])
    gW2, gb2 = inp("gW2", [D, E]), inp("gb2", [E])
    eW1 = nc.dram_tensor("eW1", [E, D, DF], BF16, kind="ExternalInput")
    eW2 = nc.dram_tensor("eW2", [E, DF, D], BF16, kind="ExternalInput")
    geW1 = nc.dram_tensor("geW1", [D, DF], BF16, kind="ExternalInput")
    geW2 = nc.dram_tensor("geW2", [DF, D], BF16, kind="ExternalInput")
    eb1, eb2 = inp("eb1", [E, DF]), inp("eb2", [E, D])
    geb1, geb2 = inp("geb1", [DF]), inp("geb2", [D])

    out_y = nc.dram_tensor("out_y", [QO, D], F32, kind="ExternalOutput")
    out_attn = nc.dram_tensor("out_attn", [H, QO, L], F32, kind="ExternalOutput")

    def wmat_into(pool, w, tag, name):
        t = pool.tile([P, DT, D], F32R, tag=tag, name=name)
        nc.sync.dma_start(
            t, w.ap().rearrange("(kt p) d -> p kt d", p=P).bitcast(F32R))
        return t

    with tile.TileContext(nc) as tc:
        with tc.tile_pool(name="consts", bufs=1) as consts:
            # ---- constants (~7KB/part) ----
            ident_f = consts.tile([P, P], F32)
            make_identity(nc, ident_f)
            ident_bf = consts.tile([P, P], BF16)
            nc.vector.tensor_copy(ident_bf, ident_f)
            ones_f = consts.tile([1, P], F32)
            nc.vector.memset(ones_f, 1.0)
            ones_r = consts.tile([1, P], F32R)
            nc.vector.tensor_copy(ones_r, ones_f)
            eps_t = consts.tile([P, 1], F32)
            nc.vector.memset(eps_t, EPS_LN)

            def colv(v, n_t):
                t = consts.tile([P, n_t], F32, tag=f"colv_{v.name}")
                nc.sync.dma_start(t, v.ap().rearrange("(t p) -> p t", p=P))
                return t

            bq_c, bk_c, bo_c = colv(bq, DT), colv(bk, DT), colv(bo, DT)
            gb1_c = colv(gb1, DT)
            geb1_c = colv(geb1, FT)
            eb1_c = consts.tile([P, E, FT], F32)
            nc.sync.dma_start(eb1_c, eb1.ap().rearrange("e (t p) -> p e t", p=P))

            bv_row = consts.tile([1, D], F32R)
            nc.sync.dma_start(bv_row, bv.ap().unsqueeze(0).bitcast(F32R))
            gb2_row = consts.tile([1, E], F32R)
            nc.sync.dma_start(gb2_row, gb2.ap().unsqueeze(0).bitcast(F32R))

            def bc(pool, v):
                t = pool.tile([P, D], F32, tag=f"bc_{v.name}")
                nc.sync.dma_start(t, _bcast_ap(v.ap()))
                return t

            gW2_s = consts.tile([P, DT, E], F32R)
            nc.sync.dma_start(
                gW2_s, gW2.ap().rearrange("(kt p) d -> p kt d", p=P).bitcast(F32R))

            with tc.tile_pool(name="perB", bufs=1) as perB:
                xt_nat = perB.tile([P, QT, D], F32)
                xtT = perB.tile([P, DT, QO], F32R)
                xtT_bf = perB.tile([P, DT, QO], BF16)
                w_sb = perB.tile([P, QT, E], F32)
                moe_acc = perB.tile([P, QT, D], F32)

                with tc.tile_pool(name="wog", bufs=1) as wog_p:
                    Wo_s = wmat_into(wog_p, Wo, "Wo", "Wo_s")
                    gW1_s = wmat_into(wog_p, gW1, "gW1", "gW1_s")
                    self_attn_and_ln1(
                        nc, tc, locals())
                    gate(nc, tc, locals())
                experts(nc, tc, locals())

    nc.compile()
    return nc


def self_attn_and_ln1(nc, tc, env):
    g = env
    consts = g["consts"]
    ident_f, ident_bf = g["ident_f"], g["ident_bf"]
    ones_r, bv_row = g["ones_r"], g["bv_row"]
    bq_c, bk_c, bo_c, eps_t = g["bq_c"], g["bk_c"], g["bo_c"], g["eps_t"]
    Wo_s = g["Wo_s"]
    x_batch, x_own = g["x_batch"], g["x_own"]
    Wq, Wk, Wv = g["Wq"], g["Wk"], g["Wv"]
    ln1_g, ln1_b = g["ln1_g"], g["ln1_b"]
    out_attn = g["out_attn"]
    xt_nat, xtT = g["xt_nat"], g["xtT"]
    xtT_bf = g["xtT_bf"]
    wmat = g["wmat_into"]
    bcf = g["bc"]

    with tc.tile_pool(name="perA", bufs=1) as perA:
        xo_nat = perA.tile([P, QT, D], F32)
        nc.sync.dma_start(xo_nat,
                          x_own.ap().rearrange("(qt p) d -> p qt d", p=P))
        kT = perA.tile([P, DT, L], F32R)
        V_bf = perA.tile([P, LT, D], BF16)
        qT = perA.tile([P, DT, QO], F32R)

        # ============ stage 1+2: xT, xoT; kT, V, qT ============
        with (
            tc.tile_pool(name="wqkv", bufs=2) as wqkv_p,
            tc.tile_pool(name="xTp", bufs=1) as xTp,
            tc.tile_pool(name="s1in", bufs=4) as s1in,
            tc.tile_pool(name="ps_tp1", bufs=3, space="PSUM") as ps_tp1,
            tc.tile_pool(name="ps_mm1", bufs=4, space="PSUM") as ps_mm1,
        ):
            xT = xTp.tile([P, DT, L], F32R)
            xoT = xTp.tile([P, DT, QO], F32R)
            for lg in range(8):
                xin = s1in.tile([P, 2, D], F32, tag="xin")
                nc.sync.dma_start(
                    xin, x_batch.ap().rearrange("(lt p) d -> p lt d", p=P)
                    [:, lg * 2:(lg + 1) * 2, :])
                for j in range(2):
                    lt = lg * 2 + j
                    ps = ps_tp1.tile([P, 4 * P], F32, tag="tp1")
                    for dt in range(DT):
                        nc.tensor.transpose(
                            ps[:, dt * P:(dt + 1) * P],
                            xin[:, j, dt * P:(dt + 1) * P], ident_f)
                    nc.vector.tensor_copy(
                        xT[:, :, lt * P:(lt + 1) * P],
                        ps.rearrange("p (a b) -> p a b", a=DT))
            for qc in range(QT):
                ps = ps_tp1.tile([P, 4 * P], F32, tag="tp1")
                for dt in range(DT):
                    nc.tensor.transpose(
                        ps[:, dt * P:(dt + 1) * P],
                        xo_nat[:, qc, dt * P:(dt + 1) * P], ident_f)
                nc.vector.tensor_copy(
                    xoT[:, :, qc * P:(qc + 1) * P],
                    ps.rearrange("p (a b) -> p a b", a=DT))

            Wk_s = wmat(wqkv_p, Wk, "wqkv", "Wk_s")
            for dt in range(DT):
                for lc in range(L // 512):
                    ps = ps_mm1.tile([P, 512], F32, tag="mm1")
                    for kt in range(DT):
                        nc.tensor.matmul(
                            ps, Wk_s[:, kt, dt * P:(dt + 1) * P],
                            xT[:, kt, lc * 512:(lc + 1) * 512],
                            start=(kt == 0), stop=(kt == DT - 1))
                    nc.vector.tensor_scalar(
                        kT[:, dt, lc * 512:(lc + 1) * 512], ps,
                        bk_c[:, dt:dt + 1], None, OP.add)
            Wv_s = wmat(wqkv_p, Wv, "wqkv", "Wv_s")
            for lt in range(LT):
                ps = ps_mm1.tile([P, 512], F32, tag="mm1")
                nc.tensor.matmul(ps, ones_r, bv_row, start=True, stop=False,
                                 skip_group_check=True)
                for kt in range(DT):
                    nc.tensor.matmul(
                        ps, xT[:, kt, lt * P:(lt + 1) * P], Wv_s[:, kt, :],
                        start=False, stop=(kt == DT - 1), skip_group_check=True)
                nc.vector.tensor_copy(V_bf[:, lt, :], ps)
            Wq_s = wmat(wqkv_p, Wq, "wqkv", "Wq_s")
            for dt in range(DT):
                ps = ps_mm1.tile([P, 512], F32, tag="mm1")
                for kt in range(DT):
                    nc.tensor.matmul(
                        ps, Wq_s[:, kt, dt * P:(dt + 1) * P], xoT[:, kt, :],
                        start=(kt == 0), stop=(kt == DT - 1))
                nc.vector.tensor_scalar(
                    qT[:, dt, :], ps, bq_c[:, dt:dt + 1], None, OP.add)

        with tc.tile_pool(name="aoT_p", bufs=1) as aoT_p:
            aoT = aoT_p.tile([P, DT, QO], F32R)

            # ============ stage 3+4: attention per head ============
            with (
                tc.tile_pool(name="exps", bufs=2) as exps_p,
                tc.tile_pool(name="attnf", bufs=2) as attnf_p,
                tc.tile_pool(name="attnb", bufs=2) as attnb_p,
                tc.tile_pool(name="expTp", bufs=2) as expT_p,
                tc.tile_pool(name="small3", bufs=8) as small3,
                tc.tile_pool(name="ps_s", bufs=1, space="PSUM") as ps_s,
                tc.tile_pool(name="ps_t3", bufs=2, space="PSUM") as ps_t3,
                tc.tile_pool(name="ps_av", bufs=1, space="PSUM") as ps_av,
            ):
                for h in range(H if "attn" not in build_nc._skip else 0):
                    hp, hr = h // 2, (h % 2) * HD
                    expT = expT_p.tile([P, LT, QO], BF16, tag="expT")
                    sums_h = small3.tile([P, QT], F32, tag="sums",
                                         name=f"sums_{h}")
                    # natural scores -> exp (f32) -> normalized attn out,
                    # interleaved with transposed-score chunks for pipelining
                    for qc in range(QT):
                        for ltc in range(qc * 2, qc * 2 + 2):
                            ps = ps_t3.tile([P, 1024], F32, tag="sct")
                            for j in range(2):
                                lt = ltc * 2 + j
                                nc.tensor.matmul(
                                    ps[:, j * 512:(j + 1) * 512],
                                    kT[hr:hr + HD, hp, lt * P:(lt + 1) * P],
                                    qT[hr:hr + HD, hp, :],
                                    start=True, stop=True)
                            nc.scalar.activation(
                                expT[:, ltc * 2:(ltc + 1) * 2, :].rearrange(
                                    "p a b -> p (a b)"),
                                ps, AF.Exp, scale=0.125)
                        qs = qT[hr:hr + HD, hp, qc * P:(qc + 1) * P]
                        halfsum = []
                        exp_sb = exps_p.tile([P, L], F32, tag="exps")
                        for half in range(2):
                            ps = ps_s.tile([P, 1024], F32, tag="sc")
                            for j in range(2):
                                lc = half * 2 + j
                                nc.tensor.matmul(
                                    ps[:, j * 512:(j + 1) * 512], qs,
                                    kT[hr:hr + HD, hp, lc * 512:(lc + 1) * 512],
                                    start=True, stop=True)
                            s = small3.tile([P, 1], F32, tag="s3")
                            nc.scalar.activation(
                                exp_sb[:, half * 1024:(half + 1) * 1024], ps,
                                AF.Exp, scale=0.125, accum_out=s)
                            halfsum.append(s)
                        nc.vector.tensor_tensor(
                            sums_h[:, qc:qc + 1], halfsum[0], halfsum[1],
                            OP.add)
                        rcp = small3.tile([P, 1], F32, tag="s3")
                        nc.vector.reciprocal(rcp, sums_h[:, qc:qc + 1])
                        if "attnout" not in build_nc._skip:
                            attn_f = attnf_p.tile([P, L], F32, tag="attnf")
                            nc.vector.tensor_scalar(attn_f, exp_sb, rcp, None,
                                                    OP.mult)
                            nc.sync.dma_start(
                                out_attn.ap()[h, qc * P:(qc + 1) * P, :],
                                attn_f)
                    # reciprocal softmax-denominator row [1, QO]
                    psr = ps_t3.tile([P, 1024], F32, tag="sct",
                                     name=f"psr_{h}")
                    for qc in range(QT):
                        nc.tensor.transpose(
                            psr[0:1, qc * P:(qc + 1) * P],
                            sums_h[:, qc:qc + 1], ident_f)
                    rcp_row = attnb_p.tile([1, QO], F32, tag="rcpr",
                                           name=f"rcpr_{h}")
                    nc.vector.reciprocal(rcp_row, psr[0:1, 0:QO])
                    rcp_bc = attnb_p.tile([HD, QO], F32, tag="rcpbc",
                                          name=f"rcpbc_{h}")
                    nc.gpsimd.partition_broadcast(rcp_bc, rcp_row)
                    # attn @ V (unnormalized), then scale columns by 1/sum
                    ps = ps_av.tile([HD, 512], F32, tag="av")
                    for lt in range(LT):
                        nc.tensor.matmul(
                            ps, V_bf[:, lt, h * HD:(h + 1) * HD],
                            expT[:, lt, :],
                            start=(lt == 0), stop=(lt == LT - 1))
                    nc.vector.tensor_tensor(
                        aoT[hr:hr + HD, hp, :], ps, rcp_bc, OP.mult)

            # ============ stage 5: out-proj, residual, LN1 ============
            with (
                tc.tile_pool(name="s5", bufs=4) as s5,
                tc.tile_pool(name="s5ln", bufs=1) as s5ln,
                tc.tile_pool(name="nxTp", bufs=1) as nxT_p,
                tc.tile_pool(name="ps_mm5", bufs=2, space="PSUM") as ps_mm5,
                tc.tile_pool(name="ps_tp5", bufs=2, space="PSUM") as ps_tp5,
            ):
                ln1g_b, ln1b_b = bcf(s5ln, ln1_g), bcf(s5ln, ln1_b)
                nxT = nxT_p.tile([P, DT, QO], F32)
                for dt in range(DT):
                    ps = ps_mm5.tile([P, 512], F32, tag="mm5")
                    for kt in range(DT):
                        nc.tensor.matmul(
                            ps, Wo_s[:, kt, dt * P:(dt + 1) * P],
                            aoT[:, kt, :],
                            start=(kt == 0), stop=(kt == DT - 1))
                    nc.vector.tensor_scalar(
                        nxT[:, dt, :], ps, bo_c[:, dt:dt + 1], None, OP.add)
                for qc in range(QT):
                    ps = ps_tp5.tile([P, 4 * P], F32, tag="tp5")
                    for dt in range(DT):
                        nc.tensor.transpose(
                            ps[:, dt * P:(dt + 1) * P],
                            nxT[:, dt, qc * P:(qc + 1) * P], ident_f)
                    r_nat = s5.tile([P, D], F32, tag="rnat")
                    nc.vector.tensor_tensor(
                        r_nat.rearrange("p (a b) -> p a b", a=DT),
                        ps.rearrange("p (a b) -> p a b", a=DT),
                        xo_nat[:, qc].rearrange("p (a b) -> p a b", a=DT),
                        OP.add)
                    stats = s5.tile([P, nc.vector.BN_STATS_DIM], F32, tag="st5")
                    nc.vector.bn_stats(stats, r_nat)
                    mv = s5.tile([P, nc.vector.BN_AGGR_DIM], F32, tag="mv5")
                    nc.vector.bn_aggr(mv, stats)
                    std = s5.tile([P, 1], F32, tag="sd5")
                    nc.scalar.activation(std, mv[:, 1:2], AF.Sqrt, bias=eps_t)
                    rstd = s5.tile([P, 1], F32, tag="rs5")
                    nc.vector.reciprocal(rstd, std)
                    tmp = s5.tile([P, D], F32, tag="tmp5")
                    nc.vector.tensor_scalar(
                        tmp, r_nat, mv[:, 0:1], rstd, OP.subtract, OP.mult)
                    nc.vector.tensor_tensor(tmp, tmp, ln1g_b, OP.mult)
                    nc.vector.tensor_tensor(xt_nat[:, qc], tmp, ln1b_b, OP.add)
                    ps2 = ps_tp5.tile([P, 4 * P], F32, tag="tp5b")
                    for dt in range(DT):
                        nc.tensor.transpose(
                            ps2[:, dt * P:(dt + 1) * P],
                            xt_nat[:, qc, dt * P:(dt + 1) * P], ident_f)
                    nc.vector.tensor_copy(
                        xtT[:, :, qc * P:(qc + 1) * P],
                        ps2.rearrange("p (a b) -> p a b", a=DT))
                    nc.vector.tensor_copy(
                        xtT_bf[:, :, qc * P:(qc + 1) * P],
                        ps2.rearrange("p (a b) -> p a b", a=DT))


def gate(nc, tc, env):
    g = env
    ones_r, gb2_row = g["ones_r"], g["gb2_row"]
    gb1_c, gW1_s, gW2_s = g["gb1_c"], g["gW1_s"], g["gW2_s"]
    xtT, w_sb = g["xtT"], g["w_sb"]

    with (
        tc.tile_pool(name="ghTp", bufs=1) as ghT_p,
        tc.tile_pool(name="s6", bufs=6) as s6,
        tc.tile_pool(name="ps_g", bufs=2, space="PSUM") as ps_g,
        tc.tile_pool(name="ps_l", bufs=2, space="PSUM") as ps_l,
    ):
        ghT = ghT_p.tile([P, DT, QO], F32R)
        for dt in range(DT):
            ps = ps_g.tile([P, 512], F32, tag="g6")
            for kt in range(DT):
                nc.tensor.matmul(
                    ps, gW1_s[:, kt, dt * P:(dt + 1) * P], xtT[:, kt, :],
                    start=(kt == 0), stop=(kt == DT - 1))
            nc.scalar.activation(
                ghT[:, dt, :], ps, AF.Gelu_apprx_tanh,
                bias=gb1_c[:, dt:dt + 1])
        for qc in range(QT):
            ps = ps_l.tile([P, E], F32, tag="l6")
            nc.tensor.matmul(ps, ones_r, gb2_row, start=True, stop=False,
                             skip_group_check=True)
            for kt in range(DT):
                nc.tensor.matmul(
                    ps, ghT[:, kt, qc * P:(qc + 1) * P], gW2_s[:, kt, :],
                    start=False, stop=(kt == DT - 1), skip_group_check=True)
            lg = s6.tile([P, E], F32, tag="lg")
            nc.vector.tensor_copy(lg, ps)
            m1 = s6.tile([P, 1], F32, tag="m1")
            nc.vector.reduce_max(m1, lg, axis=AX.X)
            mask1 = s6.tile([P, E], F32, tag="mk1")
            nc.vector.tensor_scalar(mask1, lg, m1, None, OP.is_ge)
            neg = s6.tile([P, E], F32, tag="neg")
            nc.vector.tensor_scalar(neg, mask1, -1e30, None, OP.mult)
            l2 = s6.tile([P, E], F32, tag="l2")
            nc.vector.tensor_tensor(l2, lg, neg, OP.add)
            m2 = s6.tile([P, 1], F32, tag="m2")
            nc.vector.reduce_max(m2, l2, axis=AX.X)
            mask2 = s6.tile([P, E], F32, tag="mk2")
            nc.vector.tensor_scalar(mask2, l2, m2, None, OP.is_ge)
            mask = s6.tile([P, E], F32, tag="mk")
            nc.vector.tensor_tensor(mask, mask1, mask2, OP.add)
            ex = s6.tile([P, E], F32, tag="ex")
            es = s6.tile([P, 1], F32, tag="es")
            nc.scalar.activation(ex, lg, AF.Exp, accum_out=es)
            pm = s6.tile([P, E], F32, tag="pm")
            nc.vector.tensor_tensor(pm, ex, mask, OP.mult)
            esr = s6.tile([P, 1], F32, tag="esr")
            nc.vector.reciprocal(esr, es)
            w0 = s6.tile([P, E], F32, tag="w0")
            nc.vector.tensor_scalar(w0, pm, esr, None, OP.mult)
            ws = s6.tile([P, 1], F32, tag="ws")
            nc.vector.reduce_sum(ws, w0, axis=AX.X)
            wse = s6.tile([P, 1], F32, tag="wse")
            nc.vector.tensor_scalar(wse, ws, EPS_GATE, None, OP.add)
            wsr = s6.tile([P, 1], F32, tag="wsr")
            nc.vector.reciprocal(wsr, wse)
            nc.vector.tensor_scalar(w_sb[:, qc], w0, wsr, None, OP.mult)


def experts(nc, tc, env):
    g = env
    ones_r, eps_t = g["ones_r"], g["eps_t"]
    eb1_c, geb1_c = g["eb1_c"], g["geb1_c"]
    eb2, geb2 = g["eb2"], g["geb2"]
    ln2_g, ln2_b = g["ln2_g"], g["ln2_b"]
    bcf = g["bc"]
    eW1, eW2, geW1, geW2 = g["eW1"], g["eW2"], g["geW1"], g["geW2"]
    xtT = g["xtT_bf"]
    xt_nat, w_sb, moe_acc = g["xt_nat"], g["w_sb"], g["moe_acc"]
    out_y = g["out_y"]

    with (
        tc.tile_pool(name="ebias", bufs=1) as ebias_p,
        tc.tile_pool(name="hTp", bufs=2) as hT_p,
        tc.tile_pool(name="ew1", bufs=3) as ew1_p,
        tc.tile_pool(name="ew2", bufs=3) as ew2_p,
        tc.tile_pool(name="ytmp", bufs=4) as ytmp_p,
        tc.tile_pool(name="fin", bufs=4) as fin_p,
        tc.tile_pool(name="ps_h", bufs=2, space="PSUM") as ps_h,
        tc.tile_pool(name="ps_y", bufs=4, space="PSUM") as ps_y,
    ):
        eb2_rows = ebias_p.tile([1, E, D], F32R)
        nc.sync.dma_start(eb2_rows, eb2.ap().unsqueeze(0).bitcast(F32R))
        geb2_row = ebias_p.tile([1, D], F32R)
        nc.sync.dma_start(geb2_row, geb2.ap().unsqueeze(0).bitcast(F32R))
        ln2g_b, ln2b_b = bcf(ebias_p, ln2_g), bcf(ebias_p, ln2_b)

        def expert(e):
            if e < E:
                w1, w2 = eW1.ap()[e], eW2.ap()[e]
                b1c, b2row = eb1_c[:, e], eb2_rows[:, e]
            else:
                w1, w2 = geW1.ap(), geW2.ap()
                b1c, b2row = geb1_c, geb2_row
            hT = hT_p.tile([P, FT, QO], BF16, tag="hT", name=f"hT_{e}")
            for fc in range(4):
                w1_s = ew1_p.tile([P, DT, 512], BF16, tag="w1",
                                  name=f"w1_{e}_{fc}")
                nc.sync.dma_start(
                    w1_s,
                    w1.rearrange("(kt p) f -> p kt f", p=P)
                    [:, :, fc * 512:(fc + 1) * 512])
                for j2 in range(2):
                    df0 = fc * 4 + j2 * 2
                    ps = ps_h.tile([P, 1024], F32, tag="h7",
                                   name=f"h7_{e}_{df0}")
                    for j in range(2):
                        for kt in range(DT):
                            nc.tensor.matmul(
                                ps[:, j * 512:(j + 1) * 512],
                                w1_s[:, kt,
                                     (j2 * 2 + j) * P:(j2 * 2 + j + 1) * P],
                                xtT[:, kt, :],
                                start=(kt == 0), stop=(kt == DT - 1))
                    for j in range(2):
                        df = df0 + j
                        nc.scalar.activation(
                            hT[:, df, :], ps[:, j * 512:(j + 1) * 512],
                            AF.Gelu_apprx_tanh, bias=b1c[:, df:df + 1])
            ps_ys = [ps_y.tile([P, D], F32, tag="y7", name=f"y7_{e}_{i}")
                     for i in range(QT)]
            for qc in range(QT):
                nc.tensor.matmul(ps_ys[qc], ones_r, b2row,
                                 start=True, stop=False,
                                 skip_group_check=True)
            for c4 in range(4):
                w2_s = ew2_p.tile([P, 4, D], BF16, tag="w2",
                                  name=f"w2_{e}_{c4}")
                nc.sync.dma_start(
                    w2_s,
                    w2.rearrange("(kt p) d -> p kt d", p=P)
                    [:, c4 * 4:(c4 + 1) * 4, :])
                for j4 in range(4):
                    df = c4 * 4 + j4
                    for qc in range(QT):
                        nc.tensor.matmul(
                            ps_ys[qc], hT[:, df, qc * P:(qc + 1) * P],
                            w2_s[:, j4, :],
                            start=False, stop=(df == FT - 1),
                            skip_group_check=True)
            for qc in range(QT):
                ps = ps_ys[qc]
                if e < E:
                    tmp = ytmp_p.tile([P, D], F32, tag="yt",
                                      name=f"yt_{e}_{qc}")
                    nc.vector.tensor_scalar(
                        tmp, ps, w_sb[:, qc, e:e + 1], None, OP.mult)
                    if e == 0:
                        nc.gpsimd.tensor_copy(moe_acc[:, qc], tmp)
                    else:
                        nc.gpsimd.tensor_tensor(
                            moe_acc[:, qc], moe_acc[:, qc], tmp, OP.add)
                else:
                    moe_bf = fin_p.tile([P, D], BF16, tag="mbf",
                                        name=f"mbf_{qc}")
                    nc.vector.tensor_copy(moe_bf, moe_acc[:, qc])
                    t1 = fin_p.tile([P, D], F32, tag="t1", name=f"t1_{qc}")
                    nc.vector.tensor_tensor(t1, ps, moe_bf, OP.add)
                    t2 = fin_p.tile([P, D], F32, tag="t2", name=f"t2_{qc}")
                    nc.vector.tensor_tensor(t2, t1, xt_nat[:, qc], OP.add)
                    stats = fin_p.tile([P, nc.vector.BN_STATS_DIM], F32,
                                       tag="st8", name=f"st8_{qc}")
                    nc.vector.bn_stats(stats, t2)
                    mv = fin_p.tile([P, nc.vector.BN_AGGR_DIM], F32,
                                    tag="mv8", name=f"mv8_{qc}")
                    nc.vector.bn_aggr(mv, stats)
                    std = fin_p.tile([P, 1], F32, tag="sd8", name=f"sd8_{qc}")
                    nc.scalar.activation(std, mv[:, 1:2], AF.Sqrt, bias=eps_t)
                    rstd = fin_p.tile([P, 1], F32, tag="rs8",
                                      name=f"rs8_{qc}")
                    nc.vector.reciprocal(rstd, std)
                    on = fin_p.tile([P, D], F32, tag="on", name=f"on_{qc}")
                    nc.vector.tensor_scalar(
                        on, t2, mv[:, 0:1], rstd, OP.subtract, OP.mult)
                    nc.vector.tensor_tensor(on, on, ln2g_b, OP.mult)
                    nc.vector.tensor_tensor(on, on, ln2b_b, OP.add)
                    nc.sync.dma_start(out_y.ap()[qc * P:(qc + 1) * P, :], on)

        import __main__
        skip = getattr(build_nc, "_skip", set())
        n_e = 0 if "experts" in skip else (E + 1)
        for e in range(n_e):
            expert(e)


@functools.lru_cache(maxsize=1)
def _get_nc():
    return build_nc()


_WNAMES = ["Wq", "bq", "Wk", "bk", "Wv", "bv", "Wo", "bo", "ln1_g", "ln1_b",
           "gW1", "gb1", "gW2", "gb2", "eW1", "eb1", "eW2", "eb2",
           "geW1", "geb1", "geW2", "geb2", "ln2_g", "ln2_b"]


def kernel(**inputs):
    nc = _get_nc()
    import ml_dtypes
    x = np.ascontiguousarray(np.asarray(inputs["x"], np.float32))
    shared = {}
    for n in _WNAMES:
        a = np.asarray(inputs[n], np.float32)
        if n in ("eW1", "eW2", "geW1", "geW2"):
            a = a.astype(ml_dtypes.bfloat16)
        shared[n] = np.ascontiguousarray(a)
    in_maps = []
    for c in range(NCORES):
        b, q0 = c // (NCORES // B), (c % (NCORES // B)) * QO
        m = dict(shared)
        m["x_batch"] = x[b]
        m["x_own"] = np.ascontiguousarray(x[b, q0:q0 + QO])
        in_maps.append(m)

    res = run_bass_kernel_spmd(nc, in_maps, core_ids=list(range(NCORES)))

    out = np.empty((B, L, D), np.float32)
    attn = np.empty((B, H, L, L), np.float32)
    for c in range(NCORES):
        b, q0 = c // (NCORES // B), (c % (NCORES // B)) * QO
        out[b, q0:q0 + QO] = res.results[c]["out_y"]
        attn[b, :, q0:q0 + QO, :] = res.results[c]["out_attn"]
    return out, attn, np.float32(0.0)
